# revision 31
# baseline (speedup 1.0000x reference)
"""Multi-head causal self-attention (B=4, T=2048, C=768, H=12) on 8 trn2 cores.

Sharding: core c handles batch b = c//2 and head-group hg = c%2 (6 heads each).
Each core computes its QKV projection slice, causal attention for its 6 heads,
and a partial output projection (768x2048, transposed). Host sums the two
partials per batch, transposes back, and adds b_o. No cross-core collectives.

All on-chip compute uses a transposed data layout (feature dim on partitions,
token dim on the free axis) so no per-tile transposes are needed in the
attention inner loop; softmax denominators come from an appended ones-row in
the PV matmul; normalization happens after PV via a gpsimd partition
broadcast of the reciprocal denominator. Matmuls run as float32r (full-rate
fp32 mode on the PE; plain fp32 is 4x slower).

Emission order is engine-aware (engines execute in-order): QKV chunk groups
are interleaved with the head pairs they unblock, and within a head the
scores matmul for k-block kb+1 is emitted before the PV matmuls of k-block
kb so the PE never waits on the exp (ACT) of the current block.
"""

import math
import os

import numpy as np

import concourse.bass as bass
from concourse import bacc
import concourse.mybir as mybir
import concourse.tile as tile
from concourse import bass_utils
from concourse.bass import ts
from concourse.masks import make_identity

F32 = mybir.dt.float32
F32R = mybir.dt.float32r

P = 128
T = 2048          # sequence length
C = 768           # embed dim
CS = C // P       # 6 contraction chunks
HL = 6            # heads per core
HD = 64           # head dim
O = 3 * HL * HD   # 1152 rows of the local W_attn slice (q|k|v)
OB = O // P       # 9
J = HL * HD       # 384 local y-feature dim
JS = J // P       # 3
OUTB = C // P     # 6 output row blocks
TT = T // 512     # 4 column tiles of 512


def _build_bass():
    nc = bacc.Bacc("TRN2", target_bir_lowering=False, debug=False)
    x_d = nc.dram_tensor("x", [T, C], F32, kind="ExternalInput").ap()
    w_d = nc.dram_tensor("w", [O, C], F32, kind="ExternalInput").ap()
    b_d = nc.dram_tensor("b", [O], F32, kind="ExternalInput").ap()
    wo_d = nc.dram_tensor("wo", [C, J], F32, kind="ExternalInput").ap()
    out_d = nc.dram_tensor("out", [C, T], F32, kind="ExternalOutput").ap()

    with tile.TileContext(nc) as tc, nc.allow_low_precision(
        reason="fp32r matmul pipeline; fp32 PSUM accumulation throughout"
    ):
        _emit_kernel(tc, x_d, w_d, b_d, wo_d, out_d)
    nc.compile()
    return nc


def _emit_kernel(tc, x_d, w_d, b_d, wo_d, out_d):
    nc = tc.nc
    scale = 1.0 / math.sqrt(HD)

    x_r = x_d.rearrange("(tb p) c -> p tb c", p=P)      # [128, 16, 768]
    w_r = w_d.rearrange("(ob p) c -> p ob c", p=P)      # [128, 9, 768]
    wo_r = wo_d.rearrange("(ob p) j -> p ob j", p=P)    # [128, 6, 384]
    out_r = out_d.rearrange("(ob p) t -> p ob t", p=P)  # [128, 6, 2048]

    with (
        tc.tile_pool(name="persist", bufs=1) as persist,
        tc.tile_pool(name="stage", bufs=2) as stage,
        tc.tile_pool(name="attn", bufs=2) as attn,
        tc.tile_pool(name="ps512", bufs=2, space="PSUM") as ps512,
        tc.tile_pool(name="ps_s", bufs=2, space="PSUM") as ps_s,
        tc.tile_pool(name="ps_y", bufs=2, space="PSUM") as ps_y,
    ):
        ident = persist.tile([P, P], F32)
        make_identity(nc, ident)
        identr = persist.tile([P, P], F32R)
        nc.vector.tensor_copy(identr, ident)
        ones32 = persist.tile([P, HD], F32)
        nc.vector.memset(ones32, 1.0)
        ones1 = persist.tile([1, HD], F32R)
        nc.vector.tensor_copy(ones1, ones32[0:1, :])
        bsb = persist.tile([P, OB], F32)
        nc.sync.dma_start(bsb, b_d.rearrange("(a p) -> p a", p=P))

        xt = persist.tile([P, CS, T], F32R)      # x^T   48KB/partition
        wt = persist.tile([P, CS, O], F32R)      # W^T   27KB
        wot = persist.tile([P, JS, C], F32R)     # Wo^T   9KB
        qkvT = persist.tile([P, OB, T], F32R)    # qkv^T 72KB
        yT = persist.tile([P, JS, T], F32R)      # y^T   24KB

        def transpose_pack(src_tile, n_blk, dst_fn):
            """PE-transpose n_blk [128,128] column blocks of src_tile into a
            packed PSUM tile, then one ACT copy into dst via dst_fn(psum3d)."""
            pk = ps_s.tile([P, 1024], F32, tag="s")
            for i in range(n_blk):
                nc.tensor.transpose(pk[:, ts(i, P)], src_tile[:, ts(i, P)], ident)
            dst_fn(pk[:, : n_blk * P].rearrange("p (a b) -> p a b", b=P))

        ob_order = [0, 3, 6, 1, 4, 7, 2, 5, 8]

        def emit_w(ob):
            wn = stage.tile([P, C], F32, tag="ld", name="wn", bufs=3)
            nc.sync.dma_start(wn[:, : C // 2], w_r[:, ob, : C // 2])
            nc.sync.dma_start(wn[:, C // 2 :], w_r[:, ob, C // 2 :])
            transpose_pack(
                wn, CS, lambda pk, ob=ob: nc.scalar.copy(wt[:, :, ts(ob, P)], pk)
            )

        def emit_x(tb):
            xn = stage.tile([P, C], F32, tag="ld", name="xn", bufs=3)
            nc.sync.dma_start(xn[:, : C // 2], x_r[:, tb, : C // 2])
            nc.sync.dma_start(xn[:, C // 2 :], x_r[:, tb, C // 2 :])
            transpose_pack(
                xn, CS, lambda pk, tb=tb: nc.vector.tensor_copy(xt[:, :, ts(tb, P)], pk)
            )

        def emit_wo():
            for ob in range(OUTB):
                won = stage.tile([P, C], F32, tag="ld", name="won", bufs=3)[:, :J]
                nc.sync.dma_start(won, wo_r[:, ob, :])
                transpose_pack(
                    won, JS,
                    lambda pk, ob=ob: nc.scalar.copy(wot[:, :, ts(ob, P)], pk),
                )

        def emit_qkv(ob, tts=None):
            # qkv^T[o, t] = sum_c W^T[c, o] x^T[c, t] + b[o]
            for tt in (range(TT) if tts is None else tts):
                pq = ps512.tile([P, 512], F32, tag="mm")
                for cs in range(CS):
                    nc.tensor.matmul(
                        pq,
                        wt[:, cs, ts(ob, P)],
                        xt[:, cs, ts(tt, 512)],
                        start=(cs == 0),
                        stop=(cs == CS - 1),
                    )
                nc.vector.tensor_scalar_add(
                    qkvT[:, ob, ts(tt, 512)], pq, bsb[:, ob : ob + 1]
                )

        def emit_head(hl, mid_cb=None):
            p0 = (hl % 2) * HD
            qT = qkvT[p0 : p0 + HD, hl // 2, :]       # [64, 2048] Q^T
            kT = qkvT[p0 : p0 + HD, 3 + hl // 2, :]   # [64, 2048] K^T
            vT = qkvT[p0 : p0 + HD, 6 + hl // 2, :]   # [64, 2048] V^T
            idd = identr[p0 : p0 + HD, p0 : p0 + HD]

            # V^T -> V (natural [k, d]) with an appended ones column
            vaug = attn.tile([P, T // P, HD + 1], F32R, tag="vaug", bufs=1)
            nc.vector.tensor_copy(
                vaug[:, :, HD : HD + 1], ones32[:, 0 : T // P, None]
            )
            for g in range(2):
                pk = ps512.tile([P, 512], F32R, tag="mm")
                for i in range(8):
                    nc.tensor.transpose(
                        pk[:, ts(i, HD)], vT[:, ts(g * 8 + i, P)], idd
                    )
                nc.vector.tensor_copy(
                    vaug[:, g * 8 : (g + 1) * 8, 0:HD],
                    pk.rearrange("p (a b) -> p a b", b=HD),
                )

            norm_q = []

            def flush_norms():
                while norm_q:
                    qt, yu, rd_t = norm_q.pop(0)
                    bc = ps512.tile([P, 512], F32, tag="mm", name="bc")
                    nc.tensor.matmul(
                        bc[0:HD], ones1[0:1], rd_t[0:1],
                        start=True, stop=True,
                    )
                    nc.vector.tensor_mul(
                        out=yT[p0 : p0 + HD, hl // 2, ts(qt, 512)],
                        in0=yu[0:HD],
                        in1=bc[0:HD],
                    )

            def emit_pv(kb, att, q0, lq, hf, ya_tiles):
                for qt in (2 * hf, 2 * hf + 1):
                    if kb > 4 * qt + 3:
                        continue
                    c0 = max(0, qt * 512 - q0)
                    c1 = min(lq, (qt + 1) * 512 - q0)
                    o0 = q0 + c0 - qt * 512
                    ya = ya_tiles[qt]
                    nc.tensor.matmul(
                        ya[0 : HD + 1, o0 : o0 + (c1 - c0)],
                        vaug[:, kb, :],
                        att[:, c0:c1],
                        start=(kb == 0),
                        stop=(kb == 4 * qt + 3),
                    )
                    if kb == 4 * qt + 3:
                        # normalization part A: pull y+denom out of PSUM and
                        # take the reciprocal; part B is deferred a k-block
                        # so the PE's broadcast matmul never waits on DVE
                        yu = attn.tile([P, 512], F32, tag="yu")
                        nc.vector.tensor_copy(yu[0 : HD + 1], ya[0 : HD + 1])
                        rd_t = attn.tile([P, 512], F32R, tag="yu", name="rd_t")
                        nc.vector.reciprocal(rd_t[0:1], yu[HD : HD + 1])
                        norm_q.append((qt, yu, rd_t))

            # flat (hf, kb) stage list: the scores/exp of stage i+1 are
            # emitted before the PV matmuls of stage i, across hf boundaries
            stages = [(0, kb) for kb in range(8)] + [(1, kb) for kb in range(16)]
            ya_tiles = {}
            pending = None
            for hf, kb in stages:
                if hf == 1 and kb == 0 and mid_cb is not None:
                    # drain hf0 fully (PV + norms) before the callback reads yT
                    if pending is not None:
                        emit_pv(*pending, ya_tiles)
                        pending = None
                    flush_norms()
                    mid_cb()
                for qt in (2 * hf, 2 * hf + 1):
                    if qt not in ya_tiles:
                        ya_tiles[qt] = ps_y.tile(
                            [P, 512], F32, tag="y", name=f"ya{hl}_{qt}"
                        )
                q0 = max(kb * P, hf * 1024)
                lq = (hf + 1) * 1024 - q0
                # scores^T[k, q] for k-block kb, q in [q0, q0+lq)
                sp = ps_s.tile([P, 1024], F32, tag="s")
                for j in range(0, lq, 512):
                    f = min(512, lq - j)
                    nc.tensor.matmul(
                        sp[:, j : j + f],
                        kT[:, ts(kb, P)],
                        qT[:, q0 + j : q0 + j + f],
                        start=True,
                        stop=True,
                    )
                att = attn.tile([P, 1024], F32R, tag="att", bufs=2)
                nc.scalar.activation(
                    att[:, :lq], sp[:, :lq],
                    mybir.ActivationFunctionType.Exp, scale=scale,
                )
                if kb * P == q0:
                    # diagonal block: zero out k > q entries
                    nc.gpsimd.affine_select(
                        out=att[:, :P], in_=att[:, :P],
                        compare_op=mybir.AluOpType.is_ge,
                        fill=0.0, base=0, channel_multiplier=-1,
                        pattern=[[1, P]],
                    )
                flush_norms()
                if pending is not None:
                    emit_pv(*pending, ya_tiles)
                pending = (kb, att, q0, lq, 0 if q0 + lq <= 1024 else 1)
            emit_pv(*pending, ya_tiles)
            flush_norms()

        # ---- output projection: part^T[o, t] = sum_j Wo^T[j, o] y^T[j, t]
        def emit_outproj(tts):
          for tt in tts:
            for ob in range(OUTB):
                po = ps512.tile([P, 512], F32, tag="mm")
                for js in range(JS):
                    nc.tensor.matmul(
                        po,
                        wot[:, js, ts(ob, P)],
                        yT[:, js, ts(tt, 512)],
                        start=(js == 0),
                        stop=(js == JS - 1),
                    )
                osb = stage.tile([P, C], F32, tag="ld", name="osb", bufs=3)[:, :512]
                nc.vector.tensor_copy(osb, po)
                nc.sync.dma_start(
                    out_r[:, ob, ts(2 * tt, 256)], osb[:, 0:256]
                )
                nc.sync.dma_start(
                    out_r[:, ob, ts(2 * tt + 1, 256)], osb[:, 256:512]
                )
          return

        # ---- loads/transposes and QKV group 0, interleaved at tt granularity
        for ob in ob_order[0:3]:
            emit_w(ob)
        for tb in range(T // P):
            emit_x(tb)
            if tb % 4 == 3:
                for ob in ob_order[0:3]:
                    emit_qkv(ob, tts=[tb // 4])

        # ---- interleave remaining QKV chunk groups with head pairs
        for g in range(3):
            if g > 0:
                for ob in ob_order[3 * g : 3 * g + 3]:
                    emit_w(ob)
                    emit_qkv(ob)
            emit_head(2 * g)
            if g == 2:
                emit_wo()
                emit_head(2 * g + 1, mid_cb=lambda: emit_outproj([0, 1]))
            else:
                emit_head(2 * g + 1)


        emit_outproj([2, 3])


_NC_CACHE = None
LAST_RESULTS = None


def _get_nc():
    global _NC_CACHE
    if _NC_CACHE is None:
        _NC_CACHE = _build_bass()
    return _NC_CACHE


def kernel(x, W_attn, b_attn, W_o, b_o):
    global LAST_RESULTS
    x = np.asarray(x, np.float32)
    W_attn = np.asarray(W_attn, np.float32)
    b_attn = np.asarray(b_attn, np.float32)
    W_o = np.asarray(W_o, np.float32)
    b_o = np.asarray(b_o, np.float32)

    B = x.shape[0]
    in_maps = []
    for core in range(8):
        b, hg = divmod(core, 2)
        sl = slice(hg * J, (hg + 1) * J)
        w_l = np.concatenate(
            [W_attn[sl], W_attn[768 + hg * J : 768 + (hg + 1) * J],
             W_attn[1536 + hg * J : 1536 + (hg + 1) * J]], axis=0
        )
        b_l = np.concatenate(
            [b_attn[sl], b_attn[768 + hg * J : 768 + (hg + 1) * J],
             b_attn[1536 + hg * J : 1536 + (hg + 1) * J]], axis=0
        )
        in_maps.append({
            "x": np.ascontiguousarray(x[b]),
            "w": np.ascontiguousarray(w_l),
            "b": np.ascontiguousarray(b_l),
            "wo": np.ascontiguousarray(W_o[:, sl]),
        })

    nc = _get_nc()
    LAST_RESULTS = bass_utils.run_bass_kernel_spmd(
        nc, in_maps, core_ids=list(range(8)),
        trace=bool(int(os.environ.get("KERNEL_TRACE", "0"))),
    )
    parts = [r["out"] for r in LAST_RESULTS.results]

    out = np.empty((B, T, C), np.float32)
    for b in range(B):
        out[b] = (parts[2 * b] + parts[2 * b + 1]).T + b_o
    return out


# revision 34
# speedup vs baseline: 1.0057x; 1.0057x over previous
"""Multi-head causal self-attention (B=4, T=2048, C=768, H=12) on 8 trn2 cores.

Sharding: core c handles batch b = c//2 and head-group hg = c%2 (6 heads each).
Each core computes its QKV projection slice, causal attention for its 6 heads,
and a partial output projection (768x2048, transposed). Host sums the two
partials per batch, transposes back, and adds b_o. No cross-core collectives.

All on-chip compute uses a transposed data layout (feature dim on partitions,
token dim on the free axis) so no per-tile transposes are needed in the
attention inner loop; softmax denominators come from an appended ones-row in
the PV matmul; normalization happens after PV via a gpsimd partition
broadcast of the reciprocal denominator. Matmuls run as float32r (full-rate
fp32 mode on the PE; plain fp32 is 4x slower).

Emission order is engine-aware (engines execute in-order): QKV chunk groups
are interleaved with the head pairs they unblock, and within a head the
scores matmul for k-block kb+1 is emitted before the PV matmuls of k-block
kb so the PE never waits on the exp (ACT) of the current block.
"""

import math
import os

import numpy as np

import concourse.bass as bass
from concourse import bacc
import concourse.mybir as mybir
import concourse.tile as tile
from concourse import bass_utils
from concourse.bass import ts
from concourse.masks import make_identity

F32 = mybir.dt.float32
F32R = mybir.dt.float32r

P = 128
T = 2048          # sequence length
C = 768           # embed dim
CS = C // P       # 6 contraction chunks
HL = 6            # heads per core
HD = 64           # head dim
O = 3 * HL * HD   # 1152 rows of the local W_attn slice (q|k|v)
OB = O // P       # 9
J = HL * HD       # 384 local y-feature dim
JS = J // P       # 3
OUTB = C // P     # 6 output row blocks
TT = T // 512     # 4 column tiles of 512


def _build_bass():
    nc = bacc.Bacc("TRN2", target_bir_lowering=False, debug=False)
    x_d = nc.dram_tensor("x", [T, C], F32, kind="ExternalInput").ap()
    w_d = nc.dram_tensor("w", [O, C], F32, kind="ExternalInput").ap()
    b_d = nc.dram_tensor("b", [O], F32, kind="ExternalInput").ap()
    wo_d = nc.dram_tensor("wo", [C, J], F32, kind="ExternalInput").ap()
    out_d = nc.dram_tensor("out", [C, T], F32, kind="ExternalOutput").ap()

    with tile.TileContext(nc) as tc, nc.allow_low_precision(
        reason="fp32r matmul pipeline; fp32 PSUM accumulation throughout"
    ):
        _emit_kernel(tc, x_d, w_d, b_d, wo_d, out_d)
    nc.compile()
    return nc


def _emit_kernel(tc, x_d, w_d, b_d, wo_d, out_d):
    nc = tc.nc
    scale = 1.0 / math.sqrt(HD)

    x_r = x_d.rearrange("(tb p) c -> p tb c", p=P)      # [128, 16, 768]
    w_r = w_d.rearrange("(ob p) c -> p ob c", p=P)      # [128, 9, 768]
    wo_r = wo_d.rearrange("(ob p) j -> p ob j", p=P)    # [128, 6, 384]
    out_r = out_d.rearrange("(ob p) t -> p ob t", p=P)  # [128, 6, 2048]

    with (
        tc.tile_pool(name="persist", bufs=1) as persist,
        tc.tile_pool(name="stage", bufs=2) as stage,
        tc.tile_pool(name="attn", bufs=2) as attn,
        tc.tile_pool(name="ps512", bufs=2, space="PSUM") as ps512,
        tc.tile_pool(name="ps_s", bufs=2, space="PSUM") as ps_s,
        tc.tile_pool(name="ps_y", bufs=2, space="PSUM") as ps_y,
    ):
        ident = persist.tile([P, P], F32)
        make_identity(nc, ident)
        identr = persist.tile([P, P], F32R)
        nc.vector.tensor_copy(identr, ident)
        ones32 = persist.tile([P, HD], F32)
        nc.vector.memset(ones32, 1.0)
        ones1 = persist.tile([1, HD], F32R)
        nc.vector.tensor_copy(ones1, ones32[0:1, :])
        bsb = persist.tile([P, OB], F32)
        nc.sync.dma_start(bsb, b_d.rearrange("(a p) -> p a", p=P))

        xt = persist.tile([P, CS, T], F32R)      # x^T   48KB/partition
        wt = persist.tile([P, CS, O], F32R)      # W^T   27KB
        wot = persist.tile([P, JS, C], F32R)     # Wo^T   9KB
        qkvT = persist.tile([P, OB, T], F32R)    # qkv^T 72KB
        yT = persist.tile([P, JS, T], F32R)      # y^T   24KB

        def transpose_pack(src_tile, n_blk, dst_fn):
            """PE-transpose n_blk [128,128] column blocks of src_tile into a
            packed PSUM tile, then one ACT copy into dst via dst_fn(psum3d)."""
            pk = ps_s.tile([P, 1024], F32, tag="s")
            for i in range(n_blk):
                nc.tensor.transpose(pk[:, ts(i, P)], src_tile[:, ts(i, P)], ident)
            dst_fn(pk[:, : n_blk * P].rearrange("p (a b) -> p a b", b=P))

        ob_order = [0, 3, 6, 1, 4, 7, 2, 5, 8]

        def emit_w(ob):
            wn = stage.tile([P, C], F32, tag="ld", name="wn", bufs=3)
            nc.sync.dma_start(wn[:, : C // 2], w_r[:, ob, : C // 2])
            nc.sync.dma_start(wn[:, C // 2 :], w_r[:, ob, C // 2 :])
            transpose_pack(
                wn, CS, lambda pk, ob=ob: nc.scalar.copy(wt[:, :, ts(ob, P)], pk)
            )

        def emit_x(tb):
            xn = stage.tile([P, C], F32, tag="ld", name="xn", bufs=3)
            nc.sync.dma_start(xn[:, : C // 2], x_r[:, tb, : C // 2])
            nc.sync.dma_start(xn[:, C // 2 :], x_r[:, tb, C // 2 :])
            transpose_pack(
                xn, CS, lambda pk, tb=tb: nc.vector.tensor_copy(xt[:, :, ts(tb, P)], pk)
            )

        def emit_wo():
            for ob in range(OUTB):
                won = stage.tile([P, C], F32, tag="ld", name="won", bufs=3)[:, :J]
                nc.sync.dma_start(won, wo_r[:, ob, :])
                transpose_pack(
                    won, JS,
                    lambda pk, ob=ob: nc.scalar.copy(wot[:, :, ts(ob, P)], pk),
                )

        def emit_qkv(ob, tts=None):
            # qkv^T[o, t] = sum_c W^T[c, o] x^T[c, t] + b[o]
            for tt in (range(TT) if tts is None else tts):
                pq = ps512.tile([P, 512], F32, tag="mm")
                for cs in range(CS):
                    nc.tensor.matmul(
                        pq,
                        wt[:, cs, ts(ob, P)],
                        xt[:, cs, ts(tt, 512)],
                        start=(cs == 0),
                        stop=(cs == CS - 1),
                    )
                nc.vector.tensor_scalar_add(
                    qkvT[:, ob, ts(tt, 512)], pq, bsb[:, ob : ob + 1]
                )

        def emit_head(hl, mid_cb=None, late_cb=None):
            p0 = (hl % 2) * HD
            qT = qkvT[p0 : p0 + HD, hl // 2, :]       # [64, 2048] Q^T
            kT = qkvT[p0 : p0 + HD, 3 + hl // 2, :]   # [64, 2048] K^T
            vT = qkvT[p0 : p0 + HD, 6 + hl // 2, :]   # [64, 2048] V^T
            idd = identr[p0 : p0 + HD, p0 : p0 + HD]

            # V^T -> V (natural [k, d]) with an appended ones column
            vaug = attn.tile([P, T // P, HD + 1], F32R, tag="vaug", bufs=1)
            nc.vector.tensor_copy(
                vaug[:, :, HD : HD + 1], ones32[:, 0 : T // P, None]
            )
            for g in range(2):
                pk = ps512.tile([P, 512], F32R, tag="mm")
                for i in range(8):
                    nc.tensor.transpose(
                        pk[:, ts(i, HD)], vT[:, ts(g * 8 + i, P)], idd
                    )
                nc.vector.tensor_copy(
                    vaug[:, g * 8 : (g + 1) * 8, 0:HD],
                    pk.rearrange("p (a b) -> p a b", b=HD),
                )

            norm_q = []

            def flush_norms():
                while norm_q:
                    qt, yu, rd_t = norm_q.pop(0)
                    bc = ps512.tile([P, 512], F32, tag="mm", name="bc")
                    nc.tensor.matmul(
                        bc[0:HD], ones1[0:1], rd_t[0:1],
                        start=True, stop=True,
                    )
                    nc.vector.tensor_mul(
                        out=yT[p0 : p0 + HD, hl // 2, ts(qt, 512)],
                        in0=yu[0:HD],
                        in1=bc[0:HD],
                    )

            def emit_pv(kb, att, q0, lq, hf, ya_tiles):
                for qt in (2 * hf, 2 * hf + 1):
                    if kb > 4 * qt + 3:
                        continue
                    c0 = max(0, qt * 512 - q0)
                    c1 = min(lq, (qt + 1) * 512 - q0)
                    o0 = q0 + c0 - qt * 512
                    ya = ya_tiles[qt]
                    nc.tensor.matmul(
                        ya[0 : HD + 1, o0 : o0 + (c1 - c0)],
                        vaug[:, kb, :],
                        att[:, c0:c1],
                        start=(kb == 0),
                        stop=(kb == 4 * qt + 3),
                    )
                    if kb == 4 * qt + 3:
                        # normalization part A: pull y+denom out of PSUM and
                        # take the reciprocal; part B is deferred a k-block
                        # so the PE's broadcast matmul never waits on DVE
                        yu = attn.tile([P, 512], F32, tag="yu")
                        nc.vector.tensor_copy(yu[0 : HD + 1], ya[0 : HD + 1])
                        rd_t = attn.tile([P, 512], F32R, tag="yu", name="rd_t")
                        nc.vector.reciprocal(rd_t[0:1], yu[HD : HD + 1])
                        norm_q.append((qt, yu, rd_t))

            # flat (hf, kb) stage list: the scores/exp of stage i+1 are
            # emitted before the PV matmuls of stage i, across hf boundaries
            stages = [(0, kb) for kb in range(8)] + [(1, kb) for kb in range(16)]
            ya_tiles = {}
            pending = None
            for hf, kb in stages:
                if hf == 1 and kb == 14 and late_cb is not None:
                    late_cb()
                if hf == 1 and kb == 0 and mid_cb is not None:
                    # drain hf0 fully (PV + norms) before the callback reads yT
                    if pending is not None:
                        emit_pv(*pending, ya_tiles)
                        pending = None
                    flush_norms()
                    mid_cb()
                for qt in (2 * hf, 2 * hf + 1):
                    if qt not in ya_tiles:
                        ya_tiles[qt] = ps_y.tile(
                            [P, 512], F32, tag="y", name=f"ya{hl}_{qt}"
                        )
                q0 = max(kb * P, hf * 1024)
                lq = (hf + 1) * 1024 - q0
                # scores^T[k, q] for k-block kb, q in [q0, q0+lq)
                sp = ps_s.tile([P, 1024], F32, tag="s")
                for j in range(0, lq, 512):
                    f = min(512, lq - j)
                    nc.tensor.matmul(
                        sp[:, j : j + f],
                        kT[:, ts(kb, P)],
                        qT[:, q0 + j : q0 + j + f],
                        start=True,
                        stop=True,
                    )
                att = attn.tile([P, 1024], F32R, tag="att", bufs=2)
                nc.scalar.activation(
                    att[:, :lq], sp[:, :lq],
                    mybir.ActivationFunctionType.Exp, scale=scale,
                )
                if kb * P == q0:
                    # diagonal block: zero out k > q entries
                    nc.gpsimd.affine_select(
                        out=att[:, :P], in_=att[:, :P],
                        compare_op=mybir.AluOpType.is_ge,
                        fill=0.0, base=0, channel_multiplier=-1,
                        pattern=[[1, P]],
                    )
                flush_norms()
                if pending is not None:
                    emit_pv(*pending, ya_tiles)
                pending = (kb, att, q0, lq, 0 if q0 + lq <= 1024 else 1)
            emit_pv(*pending, ya_tiles)
            flush_norms()

        # ---- output projection: part^T[o, t] = sum_j Wo^T[j, o] y^T[j, t]
        def emit_outproj(tts):
          for tt in tts:
            for ob in range(OUTB):
                po = ps512.tile([P, 512], F32, tag="mm")
                for js in range(JS):
                    nc.tensor.matmul(
                        po,
                        wot[:, js, ts(ob, P)],
                        yT[:, js, ts(tt, 512)],
                        start=(js == 0),
                        stop=(js == JS - 1),
                    )
                osb = stage.tile([P, C], F32, tag="ld", name="osb", bufs=3)[:, :512]
                nc.vector.tensor_copy(osb, po)
                nc.sync.dma_start(
                    out_r[:, ob, ts(2 * tt, 256)], osb[:, 0:256]
                )
                nc.sync.dma_start(
                    out_r[:, ob, ts(2 * tt + 1, 256)], osb[:, 256:512]
                )
          return

        # ---- loads/transposes and QKV group 0, interleaved at tt granularity
        for ob in ob_order[0:3]:
            emit_w(ob)
        for tb in range(T // P):
            emit_x(tb)
            if tb % 4 == 3:
                for ob in ob_order[0:3]:
                    emit_qkv(ob, tts=[tb // 4])

        # ---- interleave remaining QKV chunk groups with head pairs
        for g in range(3):
            if g > 0:
                for ob in ob_order[3 * g : 3 * g + 3]:
                    emit_w(ob)
                    emit_qkv(ob)
            emit_head(2 * g)
            if g == 2:
                emit_wo()
                emit_head(
                    2 * g + 1,
                    mid_cb=lambda: emit_outproj([0, 1]),
                    late_cb=lambda: emit_outproj([2]),
                )
            else:
                emit_head(2 * g + 1)


        emit_outproj([3])


_NC_CACHE = None
LAST_RESULTS = None


def _get_nc():
    global _NC_CACHE
    if _NC_CACHE is None:
        _NC_CACHE = _build_bass()
    return _NC_CACHE


def kernel(x, W_attn, b_attn, W_o, b_o):
    global LAST_RESULTS
    x = np.asarray(x, np.float32)
    W_attn = np.asarray(W_attn, np.float32)
    b_attn = np.asarray(b_attn, np.float32)
    W_o = np.asarray(W_o, np.float32)
    b_o = np.asarray(b_o, np.float32)

    B = x.shape[0]
    in_maps = []
    for core in range(8):
        b, hg = divmod(core, 2)
        sl = slice(hg * J, (hg + 1) * J)
        w_l = np.concatenate(
            [W_attn[sl], W_attn[768 + hg * J : 768 + (hg + 1) * J],
             W_attn[1536 + hg * J : 1536 + (hg + 1) * J]], axis=0
        )
        b_l = np.concatenate(
            [b_attn[sl], b_attn[768 + hg * J : 768 + (hg + 1) * J],
             b_attn[1536 + hg * J : 1536 + (hg + 1) * J]], axis=0
        )
        in_maps.append({
            "x": np.ascontiguousarray(x[b]),
            "w": np.ascontiguousarray(w_l),
            "b": np.ascontiguousarray(b_l),
            "wo": np.ascontiguousarray(W_o[:, sl]),
        })

    nc = _get_nc()
    LAST_RESULTS = bass_utils.run_bass_kernel_spmd(
        nc, in_maps, core_ids=list(range(8)),
        trace=bool(int(os.environ.get("KERNEL_TRACE", "0"))),
    )
    parts = [r["out"] for r in LAST_RESULTS.results]

    out = np.empty((B, T, C), np.float32)
    for b in range(B):
        out[b] = (parts[2 * b] + parts[2 * b + 1]).T + b_o
    return out


# revision 37
# speedup vs baseline: 1.0164x; 1.0106x over previous
"""Multi-head causal self-attention (B=4, T=2048, C=768, H=12) on 8 trn2 cores.

Sharding: core c handles batch b = c//2 and head-group hg = c%2 (6 heads each).
Each core computes its QKV projection slice, causal attention for its 6 heads,
and a partial output projection (768x2048, transposed). Host sums the two
partials per batch, transposes back, and adds b_o. No cross-core collectives.

All on-chip compute uses a transposed data layout (feature dim on partitions,
token dim on the free axis) so no per-tile transposes are needed in the
attention inner loop; softmax denominators come from an appended ones-row in
the PV matmul; normalization happens after PV via a gpsimd partition
broadcast of the reciprocal denominator. Matmuls run as float32r (full-rate
fp32 mode on the PE; plain fp32 is 4x slower).

Emission order is engine-aware (engines execute in-order): QKV chunk groups
are interleaved with the head pairs they unblock, and within a head the
scores matmul for k-block kb+1 is emitted before the PV matmuls of k-block
kb so the PE never waits on the exp (ACT) of the current block.
"""

import math
import os

import numpy as np

import concourse.bass as bass
from concourse import bacc
import concourse.mybir as mybir
import concourse.tile as tile
from concourse import bass_utils
from concourse.bass import ts
from concourse.masks import make_identity

F32 = mybir.dt.float32
F32R = mybir.dt.float32r

P = 128
T = 2048          # sequence length
C = 768           # embed dim
CS = C // P       # 6 contraction chunks
HL = 6            # heads per core
HD = 64           # head dim
O = 3 * HL * HD   # 1152 rows of the local W_attn slice (q|k|v)
OB = O // P       # 9
J = HL * HD       # 384 local y-feature dim
JS = J // P       # 3
OUTB = C // P     # 6 output row blocks
TT = T // 512     # 4 column tiles of 512


def _build_bass():
    nc = bacc.Bacc("TRN2", target_bir_lowering=False, debug=False)
    x_d = nc.dram_tensor("x", [T, C], F32, kind="ExternalInput").ap()
    w_d = nc.dram_tensor("w", [O, C], F32, kind="ExternalInput").ap()
    b_d = nc.dram_tensor("b", [O], F32, kind="ExternalInput").ap()
    wo_d = nc.dram_tensor("wo", [C, J], F32, kind="ExternalInput").ap()
    out_d = nc.dram_tensor("out", [C, T], F32, kind="ExternalOutput").ap()

    with tile.TileContext(nc) as tc, nc.allow_low_precision(
        reason="fp32r matmul pipeline; fp32 PSUM accumulation throughout"
    ):
        _emit_kernel(tc, x_d, w_d, b_d, wo_d, out_d)
    nc.compile()
    return nc


def _emit_kernel(tc, x_d, w_d, b_d, wo_d, out_d):
    nc = tc.nc
    scale = 1.0 / math.sqrt(HD)

    x_r = x_d.rearrange("(tb p) c -> p tb c", p=P)      # [128, 16, 768]
    w_r = w_d.rearrange("(ob p) c -> p ob c", p=P)      # [128, 9, 768]
    wo_r = wo_d.rearrange("(ob p) j -> p ob j", p=P)    # [128, 6, 384]
    out_r = out_d.rearrange("(ob p) t -> p ob t", p=P)  # [128, 6, 2048]

    with (
        tc.tile_pool(name="persist", bufs=1) as persist,
        tc.tile_pool(name="stage", bufs=2) as stage,
        tc.tile_pool(name="attn", bufs=2) as attn,
        tc.tile_pool(name="ps512", bufs=2, space="PSUM") as ps512,
        tc.tile_pool(name="ps_s", bufs=2, space="PSUM") as ps_s,
        tc.tile_pool(name="ps_y", bufs=2, space="PSUM") as ps_y,
    ):
        ident = persist.tile([P, P], F32)
        make_identity(nc, ident)
        identr = persist.tile([P, P], F32R)
        nc.vector.tensor_copy(identr, ident)
        ones32 = persist.tile([P, HD], F32)
        nc.vector.memset(ones32, 1.0)
        ones1 = persist.tile([1, HD], F32R)
        nc.vector.tensor_copy(ones1, ones32[0:1, :])
        bsb = persist.tile([P, OB], F32)
        nc.sync.dma_start(bsb, b_d.rearrange("(a p) -> p a", p=P))

        xt = persist.tile([P, CS, T], F32R)      # x^T   48KB/partition
        wt = persist.tile([P, CS, O], F32R)      # W^T   27KB
        wot = persist.tile([P, JS, C], F32R)     # Wo^T   9KB
        qkvT = persist.tile([P, OB, T], F32R)    # qkv^T 72KB
        yT = persist.tile([P, JS, T], F32R)      # y^T   24KB

        def transpose_pack(src_tile, n_blk, dst_fn):
            """PE-transpose n_blk [128,128] column blocks of src_tile into a
            packed PSUM tile, then one ACT copy into dst via dst_fn(psum3d)."""
            pk = ps_s.tile([P, 1024], F32, tag="s")
            for i in range(n_blk):
                nc.tensor.transpose(pk[:, ts(i, P)], src_tile[:, ts(i, P)], ident)
            dst_fn(pk[:, : n_blk * P].rearrange("p (a b) -> p a b", b=P))

        ob_order = [0, 3, 6, 1, 4, 7, 2, 5, 8]

        def emit_w(ob):
            wn = stage.tile([P, C], F32, tag="ld", name="wn", bufs=3)
            nc.sync.dma_start(wn[:, : C // 2], w_r[:, ob, : C // 2])
            nc.sync.dma_start(wn[:, C // 2 :], w_r[:, ob, C // 2 :])
            transpose_pack(
                wn, CS, lambda pk, ob=ob: nc.scalar.copy(wt[:, :, ts(ob, P)], pk)
            )

        def emit_x(tb):
            xn = stage.tile([P, C], F32, tag="ld", name="xn", bufs=3)
            nc.sync.dma_start(xn[:, : C // 2], x_r[:, tb, : C // 2])
            nc.sync.dma_start(xn[:, C // 2 :], x_r[:, tb, C // 2 :])
            transpose_pack(
                xn, CS, lambda pk, tb=tb: nc.vector.tensor_copy(xt[:, :, ts(tb, P)], pk)
            )

        def emit_wo():
            for ob in range(OUTB):
                won = stage.tile([P, C], F32, tag="ld", name="won", bufs=3)[:, :J]
                nc.sync.dma_start(won, wo_r[:, ob, :])
                transpose_pack(
                    won, JS,
                    lambda pk, ob=ob: nc.scalar.copy(wot[:, :, ts(ob, P)], pk),
                )

        def emit_qkv(ob, tts=None):
            # qkv^T[o, t] = sum_c W^T[c, o] x^T[c, t] + b[o]
            for tt in (range(TT) if tts is None else tts):
                pq = ps512.tile([P, 512], F32, tag="mm")
                for cs in range(CS):
                    nc.tensor.matmul(
                        pq,
                        wt[:, cs, ts(ob, P)],
                        xt[:, cs, ts(tt, 512)],
                        start=(cs == 0),
                        stop=(cs == CS - 1),
                    )
                nc.vector.tensor_scalar_add(
                    qkvT[:, ob, ts(tt, 512)], pq, bsb[:, ob : ob + 1]
                )

        def emit_head(hl, mid_cb=None, late_cb=None):
            p0 = (hl % 2) * HD
            qT = qkvT[p0 : p0 + HD, hl // 2, :]       # [64, 2048] Q^T
            kT = qkvT[p0 : p0 + HD, 3 + hl // 2, :]   # [64, 2048] K^T
            vT = qkvT[p0 : p0 + HD, 6 + hl // 2, :]   # [64, 2048] V^T
            idd = identr[p0 : p0 + HD, p0 : p0 + HD]

            # V^T -> V (natural [k, d]) with an appended ones column
            vaug = attn.tile([P, T // P, HD + 1], F32R, tag="vaug", bufs=1)
            nc.vector.tensor_copy(
                vaug[:, :, HD : HD + 1], ones32[:, 0 : T // P, None]
            )
            for g in range(2):
                pk = ps512.tile([P, 512], F32R, tag="mm")
                for i in range(8):
                    nc.tensor.transpose(
                        pk[:, ts(i, HD)], vT[:, ts(g * 8 + i, P)], idd
                    )
                nc.vector.tensor_copy(
                    vaug[:, g * 8 : (g + 1) * 8, 0:HD],
                    pk.rearrange("p (a b) -> p a b", b=HD),
                )

            norm_q = []

            def flush_norms():
                while norm_q:
                    qt, yu, rd_t = norm_q.pop(0)
                    bc = ps512.tile([P, 512], F32, tag="mm", name="bc")
                    nc.tensor.matmul(
                        bc[0:HD], ones1[0:1], rd_t[0:1],
                        start=True, stop=True,
                    )
                    nc.vector.tensor_mul(
                        out=yT[p0 : p0 + HD, hl // 2, ts(qt, 512)],
                        in0=yu[0:HD],
                        in1=bc[0:HD],
                    )

            def emit_pv(kb, att, q0, lq, hf, ya_tiles):
                for qt in (2 * hf, 2 * hf + 1):
                    if kb > 4 * qt + 3:
                        continue
                    c0 = max(0, qt * 512 - q0)
                    c1 = min(lq, (qt + 1) * 512 - q0)
                    o0 = q0 + c0 - qt * 512
                    ya = ya_tiles[qt]
                    nc.tensor.matmul(
                        ya[0 : HD + 1, o0 : o0 + (c1 - c0)],
                        vaug[:, kb, :],
                        att[:, c0:c1],
                        start=(kb == 0),
                        stop=(kb == 4 * qt + 3),
                    )
                    if kb == 4 * qt + 3:
                        # normalization part A: pull y+denom out of PSUM and
                        # take the reciprocal; part B is deferred a k-block
                        # so the PE's broadcast matmul never waits on DVE
                        yu = attn.tile([P, 512], F32, tag="yu")
                        nc.vector.tensor_copy(yu[0 : HD + 1], ya[0 : HD + 1])
                        rd_t = attn.tile([P, 512], F32R, tag="yu", name="rd_t")
                        nc.vector.reciprocal(rd_t[0:1], yu[HD : HD + 1])
                        norm_q.append((qt, yu, rd_t))

            # flat (hf, kb) stage list: the scores/exp of stage i+1 are
            # emitted before the PV matmuls of stage i, across hf boundaries
            stages = [(0, kb) for kb in range(8)] + [(1, kb) for kb in range(16)]
            ya_tiles = {}
            pending = None
            for hf, kb in stages:
                if hf == 1 and kb == 14 and late_cb is not None:
                    late_cb()
                if hf == 1 and kb == 0 and mid_cb is not None:
                    # drain hf0 fully (PV + norms) before the callback reads yT
                    if pending is not None:
                        emit_pv(*pending, ya_tiles)
                        pending = None
                    flush_norms()
                    mid_cb()
                for qt in (2 * hf, 2 * hf + 1):
                    if qt not in ya_tiles:
                        ya_tiles[qt] = ps_y.tile(
                            [P, 512], F32, tag="y", name=f"ya{hl}_{qt}"
                        )
                q0 = max(kb * P, hf * 1024)
                lq = (hf + 1) * 1024 - q0
                # scores^T[k, q] for k-block kb, q in [q0, q0+lq)
                sp = ps_s.tile([P, 1024], F32, tag="s")
                for j in range(0, lq, 512):
                    f = min(512, lq - j)
                    nc.tensor.matmul(
                        sp[:, j : j + f],
                        kT[:, ts(kb, P)],
                        qT[:, q0 + j : q0 + j + f],
                        start=True,
                        stop=True,
                    )
                att = attn.tile([P, 1024], F32R, tag="att", bufs=2)
                nc.scalar.activation(
                    att[:, :lq], sp[:, :lq],
                    mybir.ActivationFunctionType.Exp, scale=scale,
                )
                if kb * P == q0:
                    # diagonal block: zero out k > q entries
                    nc.gpsimd.affine_select(
                        out=att[:, :P], in_=att[:, :P],
                        compare_op=mybir.AluOpType.is_ge,
                        fill=0.0, base=0, channel_multiplier=-1,
                        pattern=[[1, P]],
                    )
                flush_norms()
                if pending is not None:
                    emit_pv(*pending, ya_tiles)
                pending = (kb, att, q0, lq, 0 if q0 + lq <= 1024 else 1)
            emit_pv(*pending, ya_tiles)
            flush_norms()

        # ---- output projection: part^T[o, t] = sum_j Wo^T[j, o] y^T[j, t]
        def emit_outproj(tts):
          for tt in tts:
            for ob in range(OUTB):
                po = ps512.tile([P, 512], F32, tag="mm")
                for js in range(JS):
                    nc.tensor.matmul(
                        po,
                        wot[:, js, ts(ob, P)],
                        yT[:, js, ts(tt, 512)],
                        start=(js == 0),
                        stop=(js == JS - 1),
                    )
                osb = stage.tile([P, C], F32, tag="ld", name="osb", bufs=3)[:, :512]
                nc.vector.tensor_copy(osb, po)
                nc.sync.dma_start(
                    out_r[:, ob, ts(2 * tt, 256)], osb[:, 0:256]
                )
                nc.sync.dma_start(
                    out_r[:, ob, ts(2 * tt + 1, 256)], osb[:, 256:512]
                )
          return

        # ---- loads/transposes and QKV group 0, interleaved at tt granularity
        for ob in ob_order[0:3]:
            emit_w(ob)
        qkv_units = []  # (ob, tt) ready once tt's x-blocks are transposed
        for tb in range(T // P):
            emit_x(tb)
            if tb % 4 == 3:
                qkv_units += [(ob, tb // 4) for ob in ob_order[0:3]]
            # drain at most one unit per x-block once available, rest at end
            if qkv_units and tb >= 3:
                ob, tt = qkv_units.pop(0)
                emit_qkv(ob, tts=[tt])
        for ob, tt in qkv_units:
            emit_qkv(ob, tts=[tt])

        # ---- interleave remaining QKV chunk groups with head pairs
        for g in range(3):
            if g > 0:
                for ob in ob_order[3 * g : 3 * g + 3]:
                    emit_w(ob)
                    emit_qkv(ob)
            emit_head(2 * g)
            if g == 2:
                emit_wo()
                emit_head(
                    2 * g + 1,
                    mid_cb=lambda: emit_outproj([0, 1]),
                    late_cb=lambda: emit_outproj([2]),
                )
            else:
                emit_head(2 * g + 1)


        emit_outproj([3])


_NC_CACHE = None
LAST_RESULTS = None


def _get_nc():
    global _NC_CACHE
    if _NC_CACHE is None:
        _NC_CACHE = _build_bass()
    return _NC_CACHE


def kernel(x, W_attn, b_attn, W_o, b_o):
    global LAST_RESULTS
    x = np.asarray(x, np.float32)
    W_attn = np.asarray(W_attn, np.float32)
    b_attn = np.asarray(b_attn, np.float32)
    W_o = np.asarray(W_o, np.float32)
    b_o = np.asarray(b_o, np.float32)

    B = x.shape[0]
    in_maps = []
    for core in range(8):
        b, hg = divmod(core, 2)
        sl = slice(hg * J, (hg + 1) * J)
        w_l = np.concatenate(
            [W_attn[sl], W_attn[768 + hg * J : 768 + (hg + 1) * J],
             W_attn[1536 + hg * J : 1536 + (hg + 1) * J]], axis=0
        )
        b_l = np.concatenate(
            [b_attn[sl], b_attn[768 + hg * J : 768 + (hg + 1) * J],
             b_attn[1536 + hg * J : 1536 + (hg + 1) * J]], axis=0
        )
        in_maps.append({
            "x": np.ascontiguousarray(x[b]),
            "w": np.ascontiguousarray(w_l),
            "b": np.ascontiguousarray(b_l),
            "wo": np.ascontiguousarray(W_o[:, sl]),
        })

    nc = _get_nc()
    LAST_RESULTS = bass_utils.run_bass_kernel_spmd(
        nc, in_maps, core_ids=list(range(8)),
        trace=bool(int(os.environ.get("KERNEL_TRACE", "0"))),
    )
    parts = [r["out"] for r in LAST_RESULTS.results]

    out = np.empty((B, T, C), np.float32)
    for b in range(B):
        out[b] = (parts[2 * b] + parts[2 * b + 1]).T + b_o
    return out
